# revision 29
# baseline (speedup 1.0000x reference)
"""Trainium2 Bass kernel for nn_Expert_13082470383822.

y = silu(depthwise_causal_conv1d(x, conv_w, K=4) + conv_b);  out = y @ W_proj.T + b_proj
x [4, 4096, 2048] fp32. Data-parallel over the 16384 (batch*seq) tokens across
8 NeuronCores (2048 tokens/core + 3-token halo).

v2: bf16 datapath. x and W are cast to bf16 on the host (halves DMA, enables
the DVE 4x perf mode for the conv taps). Per-core: channels on SBUF partitions,
conv on 512-token strips: all 4 taps as DVE scalar_tensor_tensor (tap 0 uses
op1=bypass), SiLU+conv_b on ACT writing bf16 y. Projection on the PE in bf16
(1 cycle/row) accumulating fp32 in PSUM, j-outer so ldw-opt elides redundant
LDWEIGHTS. PSUM tiles DMA straight to DRAM (no copyback op); b_proj is added
on the host.
"""

import sys

if "/opt/trn_rl_repo" not in sys.path:
    sys.path.insert(0, "/opt/trn_rl_repo")

import os

import numpy as np

if os.environ.get("BASS_LDW_OPT", "0") == "1":
    import concourse.bass_utils as _bu

    if not getattr(_bu, "_ldw_opt_patched", False):
        _orig_run_command = _bu.run_command

        def _run_command_ldw(cmd, *a, **kw):
            cmd = [
                "--enable-ldw-opt=true" if c == "--enable-ldw-opt=false" else c
                for c in cmd
            ]
            return _orig_run_command(cmd, *a, **kw)

        _bu.run_command = _run_command_ldw
        _bu._ldw_opt_patched = True

B, S, D, KW = 4, 4096, 2048, 4
NCORES = 8
T = (B * S) // NCORES  # tokens per core = 2048
KT = D // 128  # 16 channel tiles
ECH = D // 512  # 4 e-chunks of the output features
CW = 512  # conv strip width (tokens)
MS = 128  # matmul stationary strip width (tokens)
NCS = T // CW  # 4 conv strips
MPC = CW // MS  # 4 matmul strips per conv strip
JQ = 4  # j-tiles per x quarter-load

_BUILT = {}


def _build_program():
    if "nc" in _BUILT:
        return _BUILT["nc"]

    import concourse.tile as tile
    from concourse import bacc, mybir

    dt = mybir.dt
    AF = mybir.ActivationFunctionType
    ALU = mybir.AluOpType

    nc = bacc.Bacc("TRN2", target_bir_lowering=False, debug=False)
    # pre-tiled x: per (conv-strip, j-quarter): [128, 4*(CW+3)] bf16 contiguous
    xs_d = nc.declare_dram_parameter(
        "xs_t", [NCS * (KT // JQ), 128, JQ * (CW + 3)], dt.bfloat16, isOutput=False
    )
    wt = nc.declare_dram_parameter("wt", [D, D], dt.bfloat16, isOutput=False)
    cw = nc.declare_dram_parameter("cw", [128, KT * KW], dt.float32, isOutput=False)
    cb = nc.declare_dram_parameter("cb", [128, KT], dt.float32, isOutput=False)
    out = nc.declare_dram_parameter("out", [T, D], dt.bfloat16, isOutput=True)

    with tile.TileContext(nc) as tc:
        with (
            tc.tile_pool(name="consts", bufs=1) as cpool,
            tc.tile_pool(name="wpool", bufs=1) as wpool,
            tc.tile_pool(name="xpool", bufs=8) as xpool,
            tc.tile_pool(name="ypool", bufs=3) as ypool,
            tc.tile_pool(name="apool", bufs=4) as apool,
            tc.tile_pool(name="opool", bufs=8) as opool,
            tc.tile_pool(name="pspool", bufs=8, space="PSUM") as pspool,
        ):
            # warm the ACT function table before any real work
            dum = cpool.tile([1, 1], dt.float32, name="dum")
            nc.gpsimd.memset(dum[:, :], 0.0)
            nc.scalar.activation(dum[:, :], dum[:, :], AF.Silu, bias=0.0)

            cw_sb = cpool.tile([128, KT * KW], dt.float32, name="cw_sb")
            nc.gpsimd.dma_start(out=cw_sb[:, :], in_=cw[:, :])
            cb_sb = cpool.tile([128, KT], dt.float32, name="cb_sb")
            nc.gpsimd.dma_start(out=cb_sb[:, :], in_=cb[:, :])

            xq = {}

            def load_xq(c, q, eng=None):
                xt = xpool.tile([128, JQ, CW + 3], dt.bfloat16, name="xs", tag="xs")
                (eng or nc.gpsimd).dma_start(
                    out=xt[:, :, :],
                    in_=xs_d[c * (KT // JQ) + q, :, :].rearrange(
                        "p (j t) -> p j t", j=JQ
                    ),
                )
                xq[(c, q)] = xt

            def load_x(c, eng=None):
                for q in range(KT // JQ):
                    load_xq(c, q, eng)

            def conv_group(c, ys, g):
                for j in range(JQ * g, JQ * g + JQ):
                    xs = xq[(c, j // JQ)]
                    jj = j % JQ
                    acc = apool.tile([128, CW], dt.bfloat16, name="acc", tag="acc")
                    # all 4 taps on DVE (bf16 4x mode); tap 0 bypasses in1
                    nc.vector.scalar_tensor_tensor(
                        acc[:, :],
                        xs[:, jj, 0:CW],
                        cw_sb[:, j * KW : j * KW + 1],
                        xs[:, jj, 0:CW],
                        ALU.mult,
                        ALU.bypass,
                    )
                    for k in range(1, KW):
                        nc.vector.scalar_tensor_tensor(
                            acc[:, :],
                            xs[:, jj, k : k + CW],
                            cw_sb[:, j * KW + k : j * KW + k + 1],
                            acc[:, :],
                            ALU.mult,
                            ALU.add,
                        )
                    # SiLU + conv bias on ACT, full strip in one op, bf16 out
                    nc.scalar.activation(
                        ys[:, j, :],
                        acc[:, :],
                        AF.Silu,
                        bias=cb_sb[:, j : j + 1],
                    )

            # the early DMA flood starves sequencer instruction fetch, so the
            # startup loads are split across two parallel queues and ordered
            # so each tile lands just before its first use: sync carries
            # [x0q0 x0q1 W0-7 x0q2 x0q3] (j-outer strip 0 needs W[j] only at
            # matmul #4j), gpsimd carries [consts W8-15 x1 ...]
            w_sb = [None] * KT

            def load_w(js, eng):
                for j in js:
                    wj = wpool.tile([128, D], dt.bfloat16, name=f"w{j}")
                    eng.dma_start(out=wj[:, :], in_=wt[j * 128 : (j + 1) * 128, :])
                    w_sb[j] = wj

            # W alternates across the two parallel queues (even j behind
            # strip 0's x on sync, odd j behind the consts on gpsimd) so the
            # full set lands at ~0.7us/tile and never paces the j-outer
            # matmuls; x1 follows the odd W tiles, clear of the startup
            # window where DMA traffic starves sequencer instruction fetch
            load_xq(0, 0, eng=nc.sync)
            load_xq(0, 1, eng=nc.sync)
            load_w(range(1, 8, 2), eng=nc.sync)
            load_xq(0, 2, eng=nc.sync)
            load_xq(0, 3, eng=nc.sync)
            load_w(range(9, KT, 2), eng=nc.sync)
            load_w(range(0, KT, 2), eng=nc.gpsimd)
            load_x(1)

            ys0 = ypool.tile([128, KT, CW], dt.bfloat16, name="ys", tag="ys")
            for g in range(KT // JQ):
                conv_group(0, ys0, g)
            ys_strip = {0: ys0}

            for c in range(NCS):
                if c + 2 < NCS:
                    load_x(c + 2)
                if c + 1 < NCS:
                    ys_strip[c + 1] = ypool.tile(
                        [128, KT, CW], dt.bfloat16, name="ys", tag="ys"
                    )

                ys = ys_strip.pop(c)
                for m in range(MPC):
                    s = c * MPC + m
                    if c == 0:
                        # j-outer: W tiles are still streaming in during the
                        # first strip, and j-outer needs W[j] only at matmul
                        # #4j, matching the trickle schedule
                        pss = [
                            pspool.tile([128, 512], dt.float32, name="ps", tag="ps")
                            for _ in range(ECH)
                        ]
                        for j in range(KT):
                            for e in range(ECH):
                                nc.tensor.matmul(
                                    pss[e][:, :],
                                    ys[:, j, m * MS : (m + 1) * MS],
                                    w_sb[j][:, e * 512 : (e + 1) * 512],
                                    start=(j == 0),
                                    stop=(j == KT - 1),
                                )
                    else:
                        # e-outer: each e-chunk stops early so its copyback
                        # and out DMA overlap the next chunk's matmuls
                        pss = []
                        for e in range(ECH):
                            ps = pspool.tile(
                                [128, 512], dt.float32, name="ps", tag="ps"
                            )
                            for j in range(KT):
                                nc.tensor.matmul(
                                    ps[:, :],
                                    ys[:, j, m * MS : (m + 1) * MS],
                                    w_sb[j][:, e * 512 : (e + 1) * 512],
                                    start=(j == 0),
                                    stop=(j == KT - 1),
                                )
                            pss.append(ps)

                    # next strip's conv group m goes on the ACT/DVE queues
                    # BEFORE this m-tile's copybacks: its deps clear early in
                    # the strip, so the in-order ACT queue reaches the
                    # copybacks right as their accumulations stop
                    if c + 1 < NCS:
                        conv_group(c + 1, ys_strip[c + 1], m)

                    for e in range(ECH):
                        # PSUM -> SBUF bf16 on ACT; b_proj added on the host
                        os_sb = opool.tile([128, 512], dt.bfloat16, name="os", tag="os")
                        nc.scalar.copy(os_sb[:, :], pss[e][:, :])
                        nc.sync.dma_start(
                            out=out[s * MS : (s + 1) * MS, e * 512 : (e + 1) * 512],
                            in_=os_sb[:, :],
                        )

    nc.compile()
    _BUILT["nc"] = nc
    return nc


def _shard_inputs(x, conv_w, conv_b, W_proj, b_proj):
    import ml_dtypes

    bf16 = ml_dtypes.bfloat16
    wt_np = np.ascontiguousarray(W_proj.T.astype(bf16))
    cw_np = np.ascontiguousarray(
        conv_w.reshape(KT, 128, KW).transpose(1, 0, 2).reshape(128, KT * KW),
        dtype=np.float32,
    )
    cb_np = np.ascontiguousarray(conv_b.reshape(KT, 128).T, dtype=np.float32)

    x16 = x.astype(bf16)
    per_batch = S // T
    in_maps = []
    for c in range(NCORES):
        b = c // per_batch
        s0 = (c % per_batch) * T
        xp = np.zeros((T + 3, D), dtype=bf16)
        xp[3:] = x16[b, s0 : s0 + T]
        if s0 > 0:
            xp[:3] = x16[b, s0 - 3 : s0]
        xTc = xp.T  # [D, T+3]
        # [NCS, D, CW+3] sliding strips -> [NCS, 16, 128, CW+3]
        strips = np.stack([xTc[:, i * CW : i * CW + CW + 3] for i in range(NCS)])
        strips = strips.reshape(NCS, KT, 128, CW + 3)
        # -> [NCS, 4 quarters, 128, 4*(CW+3)]
        quarters = np.ascontiguousarray(
            strips.reshape(NCS, KT // JQ, JQ, 128, CW + 3).transpose(0, 1, 3, 2, 4)
        ).reshape(NCS * (KT // JQ), 128, JQ * (CW + 3))
        in_maps.append(
            {
                "xs_t": quarters,
                "wt": wt_np,
                "cw": cw_np,
                "cb": cb_np,
            }
        )
    return in_maps


def run_sharded(x, conv_w, conv_b, W_proj, b_proj, trace=False):
    """Run across the 8 cores; returns (full_out [B,S,D], BassKernelResults)."""
    from concourse.bass_utils import run_bass_kernel_spmd

    nc = _build_program()
    in_maps = _shard_inputs(x, conv_w, conv_b, W_proj, b_proj)
    try:
        res = run_bass_kernel_spmd(nc, in_maps, list(range(NCORES)), trace=trace)
    except Exception:
        # transient device wedges (NRT_EXEC_UNIT_UNRECOVERABLE) clear on retry
        res = run_bass_kernel_spmd(nc, in_maps, list(range(NCORES)), trace=trace)
    full = np.empty((B, S, D), dtype=np.float32)
    per_batch = S // T
    bp = b_proj.astype(np.float32)
    for c in range(NCORES):
        b = c // per_batch
        s0 = (c % per_batch) * T
        full[b, s0 : s0 + T] = res.results[c]["out"].astype(np.float32) + bp
    return full, res


def kernel(x, conv_w, conv_b, W_proj, b_proj):
    full, _ = run_sharded(x, conv_w, conv_b, W_proj, b_proj, trace=False)
    return full


# revision 31
# speedup vs baseline: 1.0059x; 1.0059x over previous
"""Trainium2 Bass kernel for nn_Expert_13082470383822.

y = silu(depthwise_causal_conv1d(x, conv_w, K=4) + conv_b);  out = y @ W_proj.T + b_proj
x [4, 4096, 2048] fp32. Data-parallel over the 16384 (batch*seq) tokens across
8 NeuronCores (2048 tokens/core + 3-token halo).

bf16 datapath: x and W are cast to bf16 on the host (halves DMA traffic; PE
rate is 1 cycle/row for bf16 and fp32r alike). Per-core: channels on SBUF
partitions, conv on 512-token strips with all 4 taps as DVE
scalar_tensor_tensor chains (tap 0 uses op1=bypass) and SiLU+conv_b on ACT
writing bf16 y. Projection on the PE accumulating fp32 in PSUM (j-outer for
strip 0 so W tiles can stream in behind strip 0's x; e-outer afterwards so
each PSUM bank's copyback overlaps the next accumulation). PSUM goes to SBUF
as bf16 on ACT and out via DMA; b_proj is added on the host. Startup DMAs are
split across the sync and gpsimd queues and ordered so each tile lands just
before first use — a large early burst starves sequencer instruction fetch
and delays kernel start.
"""

import sys

if "/opt/trn_rl_repo" not in sys.path:
    sys.path.insert(0, "/opt/trn_rl_repo")

import os

import numpy as np

if os.environ.get("BASS_LDW_OPT", "0") == "1":
    import concourse.bass_utils as _bu

    if not getattr(_bu, "_ldw_opt_patched", False):
        _orig_run_command = _bu.run_command

        def _run_command_ldw(cmd, *a, **kw):
            cmd = [
                "--enable-ldw-opt=true" if c == "--enable-ldw-opt=false" else c
                for c in cmd
            ]
            return _orig_run_command(cmd, *a, **kw)

        _bu.run_command = _run_command_ldw
        _bu._ldw_opt_patched = True

B, S, D, KW = 4, 4096, 2048, 4
NCORES = 8
T = (B * S) // NCORES  # tokens per core = 2048
KT = D // 128  # 16 channel tiles
ECH = D // 512  # 4 e-chunks of the output features
CW = 512  # conv strip width (tokens)
MS = 128  # matmul stationary strip width (tokens)
NCS = T // CW  # 4 conv strips
MPC = CW // MS  # 4 matmul strips per conv strip
JQ = 4  # j-tiles per x quarter-load

_BUILT = {}


def _build_program():
    if "nc" in _BUILT:
        return _BUILT["nc"]

    import concourse.tile as tile
    from concourse import bacc, mybir

    dt = mybir.dt
    AF = mybir.ActivationFunctionType
    ALU = mybir.AluOpType

    nc = bacc.Bacc("TRN2", target_bir_lowering=False, debug=False)
    # pre-tiled x: per (conv-strip, j-quarter): [128, 4*(CW+3)] bf16 contiguous
    xs_d = nc.declare_dram_parameter(
        "xs_t", [NCS * (KT // JQ), 128, JQ * (CW + 3)], dt.bfloat16, isOutput=False
    )
    wt = nc.declare_dram_parameter("wt", [D, D], dt.bfloat16, isOutput=False)
    cw = nc.declare_dram_parameter("cw", [128, KT * KW], dt.float32, isOutput=False)
    cb = nc.declare_dram_parameter("cb", [128, KT], dt.float32, isOutput=False)
    out = nc.declare_dram_parameter("out", [T, D], dt.bfloat16, isOutput=True)

    with tile.TileContext(nc) as tc:
        with (
            tc.tile_pool(name="consts", bufs=1) as cpool,
            tc.tile_pool(name="wpool", bufs=1) as wpool,
            tc.tile_pool(name="xpool", bufs=8) as xpool,
            tc.tile_pool(name="ypool", bufs=3) as ypool,
            tc.tile_pool(name="apool", bufs=4) as apool,
            tc.tile_pool(name="opool", bufs=8) as opool,
            tc.tile_pool(name="pspool", bufs=8, space="PSUM") as pspool,
        ):
            # warm the ACT function table before any real work
            dum = cpool.tile([1, 1], dt.float32, name="dum")
            nc.gpsimd.memset(dum[:, :], 0.0)
            nc.scalar.activation(dum[:, :], dum[:, :], AF.Silu, bias=0.0)

            cw_sb = cpool.tile([128, KT * KW], dt.float32, name="cw_sb")
            nc.gpsimd.dma_start(out=cw_sb[:, :], in_=cw[:, :])
            cb_sb = cpool.tile([128, KT], dt.float32, name="cb_sb")
            nc.gpsimd.dma_start(out=cb_sb[:, :], in_=cb[:, :])

            xq = {}

            def load_xq(c, q, eng=None):
                xt = xpool.tile([128, JQ, CW + 3], dt.bfloat16, name="xs", tag="xs")
                (eng or nc.gpsimd).dma_start(
                    out=xt[:, :, :],
                    in_=xs_d[c * (KT // JQ) + q, :, :].rearrange(
                        "p (j t) -> p j t", j=JQ
                    ),
                )
                xq[(c, q)] = xt

            def load_x(c, eng=None):
                for q in range(KT // JQ):
                    load_xq(c, q, eng)

            def conv_group(c, ys, g):
                for j in range(JQ * g, JQ * g + JQ):
                    xs = xq[(c, j // JQ)]
                    jj = j % JQ
                    acc = apool.tile([128, CW], dt.bfloat16, name="acc", tag="acc")
                    # all 4 taps on DVE (bf16 4x mode); tap 0 bypasses in1
                    nc.vector.scalar_tensor_tensor(
                        acc[:, :],
                        xs[:, jj, 0:CW],
                        cw_sb[:, j * KW : j * KW + 1],
                        xs[:, jj, 0:CW],
                        ALU.mult,
                        ALU.bypass,
                    )
                    for k in range(1, KW):
                        nc.vector.scalar_tensor_tensor(
                            acc[:, :],
                            xs[:, jj, k : k + CW],
                            cw_sb[:, j * KW + k : j * KW + k + 1],
                            acc[:, :],
                            ALU.mult,
                            ALU.add,
                        )
                    # SiLU + conv bias on ACT, full strip in one op, bf16 out
                    nc.scalar.activation(
                        ys[:, j, :],
                        acc[:, :],
                        AF.Silu,
                        bias=cb_sb[:, j : j + 1],
                    )

            # the early DMA flood starves sequencer instruction fetch, so the
            # startup loads are split across two parallel queues and ordered
            # so each tile lands just before its first use: sync carries
            # [x0q0 x0q1 W0-7 x0q2 x0q3] (j-outer strip 0 needs W[j] only at
            # matmul #4j), gpsimd carries [consts W8-15 x1 ...]
            w_sb = [None] * KT

            def load_w(js, eng):
                for j in js:
                    wj = wpool.tile([128, D], dt.bfloat16, name=f"w{j}")
                    eng.dma_start(out=wj[:, :], in_=wt[j * 128 : (j + 1) * 128, :])
                    w_sb[j] = wj

            # W alternates across the two parallel queues (even j behind
            # strip 0's x on sync, odd j behind the consts on gpsimd) so the
            # full set lands at ~0.7us/tile and never paces the j-outer
            # matmuls; x1 follows the odd W tiles, clear of the startup
            # window where DMA traffic starves sequencer instruction fetch
            load_x(0, eng=nc.sync)
            load_w(range(1, KT, 2), eng=nc.sync)
            load_w(range(0, KT, 2), eng=nc.gpsimd)
            load_x(1)

            ys0 = ypool.tile([128, KT, CW], dt.bfloat16, name="ys", tag="ys")
            for g in range(KT // JQ):
                conv_group(0, ys0, g)
            ys_strip = {0: ys0}

            for c in range(NCS):
                if c + 2 < NCS:
                    load_x(c + 2)
                if c + 1 < NCS:
                    ys_strip[c + 1] = ypool.tile(
                        [128, KT, CW], dt.bfloat16, name="ys", tag="ys"
                    )

                ys = ys_strip.pop(c)
                for m in range(MPC):
                    s = c * MPC + m
                    if c == 0:
                        # j-outer: W tiles are still streaming in during the
                        # first strip, and j-outer needs W[j] only at matmul
                        # #4j, matching the trickle schedule
                        pss = [
                            pspool.tile([128, 512], dt.float32, name="ps", tag="ps")
                            for _ in range(ECH)
                        ]
                        for j in range(KT):
                            for e in range(ECH):
                                nc.tensor.matmul(
                                    pss[e][:, :],
                                    ys[:, j, m * MS : (m + 1) * MS],
                                    w_sb[j][:, e * 512 : (e + 1) * 512],
                                    start=(j == 0),
                                    stop=(j == KT - 1),
                                )
                    else:
                        # e-outer: each e-chunk stops early so its copyback
                        # and out DMA overlap the next chunk's matmuls
                        pss = []
                        for e in range(ECH):
                            ps = pspool.tile(
                                [128, 512], dt.float32, name="ps", tag="ps"
                            )
                            for j in range(KT):
                                nc.tensor.matmul(
                                    ps[:, :],
                                    ys[:, j, m * MS : (m + 1) * MS],
                                    w_sb[j][:, e * 512 : (e + 1) * 512],
                                    start=(j == 0),
                                    stop=(j == KT - 1),
                                )
                            pss.append(ps)

                    # next strip's conv group m goes on the ACT/DVE queues
                    # BEFORE this m-tile's copybacks: its deps clear early in
                    # the strip, so the in-order ACT queue reaches the
                    # copybacks right as their accumulations stop
                    if c + 1 < NCS:
                        conv_group(c + 1, ys_strip[c + 1], m)

                    for e in range(ECH):
                        # PSUM -> SBUF bf16 on ACT; b_proj added on the host
                        os_sb = opool.tile([128, 512], dt.bfloat16, name="os", tag="os")
                        nc.scalar.copy(os_sb[:, :], pss[e][:, :])
                        nc.sync.dma_start(
                            out=out[s * MS : (s + 1) * MS, e * 512 : (e + 1) * 512],
                            in_=os_sb[:, :],
                        )

    nc.compile()
    _BUILT["nc"] = nc
    return nc


def _shard_inputs(x, conv_w, conv_b, W_proj, b_proj):
    import ml_dtypes

    bf16 = ml_dtypes.bfloat16
    wt_np = np.ascontiguousarray(W_proj.T.astype(bf16))
    cw_np = np.ascontiguousarray(
        conv_w.reshape(KT, 128, KW).transpose(1, 0, 2).reshape(128, KT * KW),
        dtype=np.float32,
    )
    cb_np = np.ascontiguousarray(conv_b.reshape(KT, 128).T, dtype=np.float32)

    x16 = x.astype(bf16)
    per_batch = S // T
    in_maps = []
    for c in range(NCORES):
        b = c // per_batch
        s0 = (c % per_batch) * T
        xp = np.zeros((T + 3, D), dtype=bf16)
        xp[3:] = x16[b, s0 : s0 + T]
        if s0 > 0:
            xp[:3] = x16[b, s0 - 3 : s0]
        xTc = xp.T  # [D, T+3]
        # [NCS, D, CW+3] sliding strips -> [NCS, 16, 128, CW+3]
        strips = np.stack([xTc[:, i * CW : i * CW + CW + 3] for i in range(NCS)])
        strips = strips.reshape(NCS, KT, 128, CW + 3)
        # -> [NCS, 4 quarters, 128, 4*(CW+3)]
        quarters = np.ascontiguousarray(
            strips.reshape(NCS, KT // JQ, JQ, 128, CW + 3).transpose(0, 1, 3, 2, 4)
        ).reshape(NCS * (KT // JQ), 128, JQ * (CW + 3))
        in_maps.append(
            {
                "xs_t": quarters,
                "wt": wt_np,
                "cw": cw_np,
                "cb": cb_np,
            }
        )
    return in_maps


def run_sharded(x, conv_w, conv_b, W_proj, b_proj, trace=False):
    """Run across the 8 cores; returns (full_out [B,S,D], BassKernelResults)."""
    from concourse.bass_utils import run_bass_kernel_spmd

    nc = _build_program()
    in_maps = _shard_inputs(x, conv_w, conv_b, W_proj, b_proj)
    try:
        res = run_bass_kernel_spmd(nc, in_maps, list(range(NCORES)), trace=trace)
    except Exception:
        # transient device wedges (NRT_EXEC_UNIT_UNRECOVERABLE) clear on retry
        res = run_bass_kernel_spmd(nc, in_maps, list(range(NCORES)), trace=trace)
    full = np.empty((B, S, D), dtype=np.float32)
    per_batch = S // T
    bp = b_proj.astype(np.float32)
    for c in range(NCORES):
        b = c // per_batch
        s0 = (c % per_batch) * T
        full[b, s0 : s0 + T] = res.results[c]["out"].astype(np.float32) + bp
    return full, res


def kernel(x, conv_w, conv_b, W_proj, b_proj):
    full, _ = run_sharded(x, conv_w, conv_b, W_proj, b_proj, trace=False)
    return full
